# revision 21
# baseline (speedup 1.0000x reference)
"""Trainium2 Bass kernel for nn_ExpertClassifierBank.

Computes, for pooled [B,K,D], expert weights [E,C,D], indices [K], log_scales [E]:
    x = l2norm(pooled, axis=-1)
    w = l2norm(weights[idx], axis=-1)
    out[b,k,c] = min(exp(log_scales[idx[k]]), 100) * dot(x[b,k], w[k,c])

Sharding: data-parallel over batch B across 8 NeuronCores (512 rows each);
the gathered expert weight bank is replicated.

Device algorithm per core (B_loc=512, K=8, D=1024=8x128, C=100):
  - host pre-transposes x and w to [k, d-part, j, *] bf16 tiles so the
    contraction dim d sits on SBUF partitions; host also folds the tiny
    per-expert constants into rwt[c,k] = min(exp(ls_k),100)/||w_k,c||
    (K*C values, same indexing-adjacent class as the expert gather).
  - main matmuls produce lg[k] = w_k^T x in PSUM; the drain to SBUF applies
    rwt as a per-partition scale (ACT Copy); the x-normalizer
    f = rsqrt(||x||^2) is broadcast over C partitions by a tiny selector
    matmul (selc4, f32r) and multiplied in on the DVE with bf16 output
    (halves output traffic).
  - row sums-of-squares: bf16 squares (split DVE 2x-packed / ACT) stream
    through the PE with one-hot k-slot selectors into a [4,BLOC] PSUM
    accumulator per half; recip+sqrt give f.
  - one ACT table set (sqrt_and_others: Square/Sqrt/Copy) loaded at t~0 by
    a dummy op.
  - x streams on the sync HWDGE queue from t=0 (k0 quartered, k1-3 halved
    for fast rampup); w + consts lead the gpsimd SWDGE queue; outputs k<4
    go out mid-kernel on gpsimd, k>=4 on the sync queue tail.  Per k the
    PE runs mains j0-3, ss j0-3, mains j4-7, ss j4-7; drains run one k
    behind, the f/output stage four k behind (gated by the half's f tile).
"""

import time

import numpy as np
import ml_dtypes

import concourse.bass as bass
import concourse.mybir as mybir
import concourse.tile as tile
from concourse import bacc
from concourse.bass_utils import run_bass_kernel_spmd

N_CORES = 8
B, K, D, C, E = 4096, 8, 1024, 100, 16
BLOC = B // N_CORES  # 512
P = 128
DC = D // P  # 8 d-chunks
HALF = 4  # k-batch size for the f pipeline

F32 = mybir.dt.float32
F32R = mybir.dt.float32r
BF16 = mybir.dt.bfloat16
AF = mybir.ActivationFunctionType
NPBF16 = ml_dtypes.bfloat16

_CACHE = {}

LAST_RESULT = None
LAST_WALL_NS = None


def _build():
    nc = bacc.Bacc(
        "TRN2", target_bir_lowering=False, debug=False, num_devices=N_CORES
    )

    xt = nc.dram_tensor("xt", [K, P, DC, BLOC], BF16, kind="ExternalInput").ap()
    wt = nc.dram_tensor("wt", [K, P, DC, C], BF16, kind="ExternalInput").ap()
    rwt = nc.dram_tensor("rwt", [C, K], F32, kind="ExternalInput").ap()
    selk4 = nc.dram_tensor("selk4", [P, HALF, HALF], BF16, kind="ExternalInput").ap()
    selc4 = nc.dram_tensor("selc4", [HALF, HALF, C], F32R, kind="ExternalInput").ap()
    out = nc.dram_tensor("out", [K, C, BLOC], BF16, kind="ExternalOutput").ap()

    with tile.TileContext(nc) as tc:
        with (
            tc.tile_pool(name="const", bufs=1) as cpool,
            tc.tile_pool(name="xres", bufs=K) as xpool,
            tc.tile_pool(name="x2", bufs=3) as x2pool,
            tc.tile_pool(name="wres", bufs=K) as wpool,
            tc.tile_pool(name="small", bufs=1) as spool,
            tc.tile_pool(name="lgs", bufs=5) as lgspool,
            tc.tile_pool(name="osb", bufs=3) as opool,
            tc.tile_pool(name="fx", bufs=2) as fxpool,
        ):
            # dummy first ACT op: pulls the (single) table set load to t~0
            dum = spool.tile([1, 1], F32, name="dum")
            nc.vector.memset(dum[:], 1.0)
            dum2 = spool.tile([1, 1], F32, name="dum2")
            nc.scalar.activation(dum2[:], dum[:], AF.Square)

            # ---- w + consts on the gpsimd SWDGE queue (w0 first: it
            # gates the first main matmuls) ----
            w_sbs = [None] * K

            def gp_w(k):
                w1 = wpool.tile([P, DC, C], BF16, tag="w", name=f"w{k}")
                nc.gpsimd.dma_start(w1[:], wt[k])
                w_sbs[k] = w1

            gp_w(0)
            selk4_sb = cpool.tile([P, HALF, HALF], BF16)
            nc.gpsimd.dma_start(selk4_sb[:], selk4[:])
            gp_w(1)
            rwt_sb = cpool.tile([C, K], F32)
            nc.gpsimd.dma_start(rwt_sb[:], rwt[:])
            gp_w(2)
            selc4_sb = cpool.tile([HALF, HALF, C], F32R)
            nc.gpsimd.dma_start(selc4_sb[:], selc4[:])
            for k in range(3, K):
                gp_w(k)

            # ---- x on the sync HWDGE queue: k0 halved, rest whole-k
            # (fewer pushes = faster descriptor issue = faster DMA ramp) ----
            x_sbs = []
            for k in range(K):
                x_sb = xpool.tile([P, DC, BLOC], BF16, tag="x", name=f"x{k}")
                nsplit = 2 if k == 0 else 1
                step = DC // nsplit
                for q in range(nsplit):
                    nc.sync.dma_start(
                        x_sb[:, q * step : (q + 1) * step],
                        xt[k][:, q * step : (q + 1) * step],
                    )
                x_sbs.append(x_sb)

            with (
                tc.tile_pool(name="pss", bufs=2, space="PSUM") as pss,
                tc.tile_pool(name="plog", bufs=3, space="PSUM") as plog,
                tc.tile_pool(name="pfb", bufs=3, space="PSUM") as pfb,
            ):
                sss = []
                fx_sbs = []
                lg_tiles = {}
                o_done = set()

                def emit_drain(k):
                    lgs = lgspool.tile([C, BLOC], F32, tag="lgs",
                                       name=f"lgs{k}")
                    nc.scalar.activation(
                        lgs[:], lg_tiles[k][:], AF.Copy,
                        scale=rwt_sb[:, k : k + 1],
                    )
                    lg_tiles[k] = lgs

                def emit_output_stage(k):
                    """f-broadcast matmul + mul + out DMA (fx half ready)."""
                    half, i = divmod(k, HALF)
                    fb = pfb.tile([C, BLOC], F32, tag="fb", name=f"fb{k}")
                    nc.tensor.matmul(
                        fb[:],
                        lhsT=selc4_sb[:, i, :],
                        rhs=fx_sbs[half][:],
                        start=True, stop=True,
                        skip_group_check=True,
                    )
                    o_sb = opool.tile([C, BLOC], BF16, tag="osb", name=f"o{k}")
                    nc.vector.tensor_mul(o_sb[:], lg_tiles[k][:], fb[:])
                    if k < HALF:
                        nc.gpsimd.dma_start(out[k], o_sb[:])
                    else:
                        nc.sync.dma_start(out[k], o_sb[:])
                    o_done.add(k)

                for k in range(K):
                    half, i = divmod(k, HALF)
                    if i == 0:
                        ss = pss.tile([HALF, BLOC], F32, tag="ss",
                                      name=f"ss{half}")
                        sss.append(ss)
                    ss = sss[half]

                    # ---- squares: bf16; ~10 pairs on ACT, 22 on DVE ----
                    x2 = x2pool.tile([P, DC, BLOC], BF16, tag="x2",
                                     name=f"x2_{k}")
                    for p in range(DC // 2):
                        src = x_sbs[k][:, 2 * p : 2 * p + 2]
                        dst = x2[:, 2 * p : 2 * p + 2]
                        on_act = (p == 1) or (p == 3 and k in (1, 3))
                        if on_act:
                            nc.scalar.activation(dst, src, AF.Square)
                        else:
                            nc.vector.tensor_mul(dst, src, src)

                    # ---- PE: mains j0-3, ss j0-3, mains j4-7, ss j4-7 ----
                    lg = plog.tile([C, BLOC], F32, tag="lg", name=f"lg{k}")

                    def mains(j0, j1):
                        for j in range(j0, j1):
                            nc.tensor.matmul(
                                lg[:], lhsT=w_sbs[k][:, j, :],
                                rhs=x_sbs[k][:, j],
                                start=(j == 0), stop=(j == DC - 1),
                                skip_group_check=True,
                            )

                    def ssmm(j0, j1):
                        for j in range(j0, j1):
                            nc.tensor.matmul(
                                ss[:],
                                lhsT=selk4_sb[:, i, :],
                                rhs=x2[:, j],
                                start=(i == 0 and j == 0),
                                stop=(i == HALF - 1 and j == DC - 1),
                                skip_group_check=True,
                            )

                    # whole-k blocks: 2 weight-set transitions per k instead
                    # of 4; k=0 runs mains first (they gate only on raw DMA)
                    if k == 0:
                        mains(0, DC // 2)
                        ssmm(0, DC // 2)
                        mains(DC // 2, DC)
                        ssmm(DC // 2, DC)
                    else:
                        ssmm(0, DC)
                        mains(0, DC)
                    lg_tiles[k] = lg

                    if k >= 1:
                        emit_drain(k - 1)

                    # half closed: f = sqrt(1/ss)
                    if i == HALF - 1:
                        recx = fxpool.tile([HALF, BLOC], F32, tag="recx",
                                           name=f"recx{half}")
                        scr = fxpool.tile([HALF, BLOC], F32, tag="rscr",
                                          name=f"rscr{half}")
                        nc.vector.reciprocal_approx_accurate(
                            recx[:], ss[:], scr[:]
                        )
                        fx = fxpool.tile([HALF, BLOC], F32R, tag="fx",
                                         name=f"fx{half}")
                        nc.scalar.activation(fx[:], recx[:], AF.Sqrt)
                        fx_sbs.append(fx)

                    if k >= HALF:
                        emit_output_stage(k - HALF)

                emit_drain(K - 1)
                for k in range(K):
                    if k not in o_done:
                        emit_output_stage(k)

    nc.compile()
    return nc


def _host_prep(pooled, active_expert_indices, weights, log_scales):
    idx = np.asarray(active_expert_indices).astype(np.int64)
    pooled = np.asarray(pooled, dtype=np.float32)
    weights = np.asarray(weights, dtype=np.float32)
    log_scales = np.asarray(log_scales, dtype=np.float32)

    # x: [B,K,D] -> bf16 -> per-core [K, P, DC, BLOC]  (k, d, j, b)
    pb = pooled.astype(NPBF16)
    xt_all = np.ascontiguousarray(
        pb.reshape(N_CORES, BLOC, K, DC, P).transpose(0, 2, 4, 3, 1)
    )
    # w: gather -> bf16 matmul layout; rwt folds scale / ||w|| (match the
    # bf16-rounded w the device multiplies with)
    wg = weights[idx].astype(NPBF16)  # [K, C, D]
    wt = np.ascontiguousarray(wg.reshape(K, C, DC, P).transpose(0, 3, 2, 1))
    s = np.minimum(np.exp(log_scales[idx]), 100.0).astype(np.float32)
    wn = np.sqrt((wg.astype(np.float32) ** 2).sum(-1))  # [K, C]
    rwt = np.ascontiguousarray((s[:, None] / np.maximum(wn, 1e-12)).T)

    selk4 = np.zeros((P, HALF, HALF), NPBF16)
    for i in range(HALF):
        selk4[:, i, i] = 1.0
    selc4 = np.zeros((HALF, HALF, C), np.float32)
    for i in range(HALF):
        selc4[i, i, :] = 1.0

    shared = {"wt": wt, "rwt": rwt, "selk4": selk4, "selc4": selc4}
    return [dict(shared, xt=np.ascontiguousarray(xt_all[co]))
            for co in range(N_CORES)]


def kernel(pooled, active_expert_indices, weights, log_scales):
    global LAST_RESULT, LAST_WALL_NS
    if "nc" not in _CACHE:
        _CACHE["nc"] = _build()
    nc = _CACHE["nc"]

    in_maps = _host_prep(pooled, active_expert_indices, weights, log_scales)

    t0 = time.perf_counter_ns()
    res = run_bass_kernel_spmd(nc, in_maps, core_ids=list(range(N_CORES)))
    LAST_WALL_NS = time.perf_counter_ns() - t0
    LAST_RESULT = res

    full = np.stack([res.results[co]["out"] for co in range(N_CORES)])
    return np.ascontiguousarray(
        full.transpose(0, 3, 1, 2).reshape(B, K, C)
    ).astype(np.float32)


# revision 24
# speedup vs baseline: 1.0124x; 1.0124x over previous
"""Trainium2 Bass kernel for nn_ExpertClassifierBank.

Computes, for pooled [B,K,D], expert weights [E,C,D], indices [K], log_scales [E]:
    x = l2norm(pooled, axis=-1)
    w = l2norm(weights[idx], axis=-1)
    out[b,k,c] = min(exp(log_scales[idx[k]]), 100) * dot(x[b,k], w[k,c])

Sharding: data-parallel over batch B across 8 NeuronCores (512 rows each);
the gathered expert weight bank is replicated.

Device algorithm per core (B_loc=512, K=8, D=1024=8x128, C=100):
  - host pre-transposes x and w to [k, d-part, j, *] bf16 tiles so the
    contraction dim d sits on SBUF partitions; host also folds the tiny
    per-expert constants into rwt[c,k] = min(exp(ls_k),100)/||w_k,c||
    (K*C values, same indexing-adjacent class as the expert gather).
  - main matmuls produce lg[k] = w_k^T x in PSUM; the drain to SBUF applies
    rwt as a per-partition scale (ACT Copy); the x-normalizer
    f = rsqrt(||x||^2) is broadcast over C partitions by a tiny selector
    matmul (selc4, f32r) and multiplied in on the DVE with bf16 output
    (halves output traffic).
  - row sums-of-squares: bf16 squares (split DVE 2x-packed / ACT) stream
    through the PE with one-hot k-slot selectors into a [4,BLOC] PSUM
    accumulator per half; recip+sqrt give f.
  - one ACT table set (sqrt_and_others: Square/Sqrt/Copy) loaded at t~0 by
    a dummy op.
  - x streams on the sync HWDGE queue from t=0 (k0 quartered, k1-3 halved
    for fast rampup); w + consts lead the gpsimd SWDGE queue; outputs k<4
    go out mid-kernel on gpsimd, k>=4 on the sync queue tail.  Per k the
    PE runs mains j0-3, ss j0-3, mains j4-7, ss j4-7; drains run one k
    behind, the f/output stage four k behind (gated by the half's f tile).
"""

import time

import numpy as np
import ml_dtypes

import concourse.bass as bass
import concourse.mybir as mybir
import concourse.tile as tile
from concourse import bacc
from concourse.bass_utils import run_bass_kernel_spmd

N_CORES = 8
B, K, D, C, E = 4096, 8, 1024, 100, 16
BLOC = B // N_CORES  # 512
P = 128
DC = D // P  # 8 d-chunks
HALF = 4  # k-batch size for the f pipeline

F32 = mybir.dt.float32
F32R = mybir.dt.float32r
BF16 = mybir.dt.bfloat16
AF = mybir.ActivationFunctionType
NPBF16 = ml_dtypes.bfloat16

_CACHE = {}

LAST_RESULT = None
LAST_WALL_NS = None


def _build():
    nc = bacc.Bacc(
        "TRN2", target_bir_lowering=False, debug=False, num_devices=N_CORES
    )

    xt = nc.dram_tensor("xt", [K, P, DC, BLOC], BF16, kind="ExternalInput").ap()
    wt = nc.dram_tensor("wt", [K, P, DC, C], BF16, kind="ExternalInput").ap()
    rwt = nc.dram_tensor("rwt", [C, K], F32, kind="ExternalInput").ap()
    selk4 = nc.dram_tensor("selk4", [P, HALF, HALF], BF16, kind="ExternalInput").ap()
    selc4 = nc.dram_tensor("selc4", [HALF, HALF, C], F32R, kind="ExternalInput").ap()
    out = nc.dram_tensor("out", [K, C, BLOC], BF16, kind="ExternalOutput").ap()

    with tile.TileContext(nc) as tc:
        with (
            tc.tile_pool(name="const", bufs=1) as cpool,
            tc.tile_pool(name="xres", bufs=K) as xpool,
            tc.tile_pool(name="x2", bufs=3) as x2pool,
            tc.tile_pool(name="wres", bufs=3) as wpool,
            tc.tile_pool(name="small", bufs=1) as spool,
            tc.tile_pool(name="lgs", bufs=5) as lgspool,
            tc.tile_pool(name="osb", bufs=3) as opool,
            tc.tile_pool(name="fx", bufs=2) as fxpool,
        ):
            # dummy first ACT op: pulls the (single) table set load to t~0
            dum = spool.tile([1, 1], F32, name="dum")
            nc.vector.memset(dum[:], 1.0)
            dum2 = spool.tile([1, 1], F32, name="dum2")
            nc.scalar.activation(dum2[:], dum[:], AF.Square)

            # ---- w + consts on the gpsimd SWDGE queue (w0 first: it
            # gates the first main matmuls) ----
            w_sbs = [None] * K

            def gp_w(k):
                w1 = wpool.tile([P, DC, C], BF16, tag="w", name=f"w{k}")
                nc.gpsimd.dma_start(w1[:], wt[k])
                w_sbs[k] = w1

            gp_w(0)
            selk4_sb = cpool.tile([P, HALF, HALF], BF16)
            nc.gpsimd.dma_start(selk4_sb[:], selk4[:])
            gp_w(1)
            rwt_sb = cpool.tile([C, K], F32)
            nc.gpsimd.dma_start(rwt_sb[:], rwt[:])
            gp_w(2)
            selc4_sb = cpool.tile([HALF, HALF, C], F32R)
            nc.gpsimd.dma_start(selc4_sb[:], selc4[:])
            for k in range(3, K):
                gp_w(k)

            # ---- x on the sync HWDGE queue: k0 halved, rest whole-k
            # (fewer pushes = faster descriptor issue = faster DMA ramp) ----
            x_sbs = []
            for k in range(K):
                x_sb = xpool.tile([P, DC, BLOC], BF16, tag="x", name=f"x{k}")
                nsplit = 2 if k == 0 else 1
                step = DC // nsplit
                for q in range(nsplit):
                    nc.sync.dma_start(
                        x_sb[:, q * step : (q + 1) * step],
                        xt[k][:, q * step : (q + 1) * step],
                    )
                x_sbs.append(x_sb)

            with (
                tc.tile_pool(name="pss", bufs=2, space="PSUM") as pss,
                tc.tile_pool(name="plog", bufs=3, space="PSUM") as plog,
                tc.tile_pool(name="pfb", bufs=3, space="PSUM") as pfb,
            ):
                sss = []
                fx_sbs = []
                lg_tiles = {}
                o_done = set()

                # HAM warmup: ~1.3us of junk matmuls on w0 as soon as it
                # lands, so the PE clock gate opens before the real stream
                wu = pfb.tile([HALF, 400], F32, tag="fb", name="warmup")
                for _ in range(4):
                    nc.tensor.matmul(
                        wu[:], lhsT=selk4_sb[:, 0, :],
                        rhs=w_sbs[0][:, :4],
                        start=True, stop=True,
                        skip_group_check=True,
                    )

                def emit_drain(k):
                    lgs = lgspool.tile([C, BLOC], F32, tag="lgs",
                                       name=f"lgs{k}")
                    nc.scalar.activation(
                        lgs[:], lg_tiles[k][:], AF.Copy,
                        scale=rwt_sb[:, k : k + 1],
                    )
                    lg_tiles[k] = lgs

                def emit_output_stage(k):
                    """f-broadcast matmul + mul + out DMA (fx half ready)."""
                    half, i = divmod(k, HALF)
                    fb = pfb.tile([C, BLOC], F32, tag="fb", name=f"fb{k}")
                    nc.tensor.matmul(
                        fb[:],
                        lhsT=selc4_sb[:, i, :],
                        rhs=fx_sbs[half][:],
                        start=True, stop=True,
                        skip_group_check=True,
                    )
                    o_sb = opool.tile([C, BLOC], BF16, tag="osb", name=f"o{k}")
                    nc.vector.tensor_mul(o_sb[:], lg_tiles[k][:], fb[:])
                    if k < HALF:
                        nc.gpsimd.dma_start(out[k], o_sb[:])
                    else:
                        nc.sync.dma_start(out[k], o_sb[:])
                    o_done.add(k)

                for k in range(K):
                    half, i = divmod(k, HALF)
                    if i == 0:
                        ss = pss.tile([HALF, BLOC], F32, tag="ss",
                                      name=f"ss{half}")
                        sss.append(ss)
                    ss = sss[half]

                    # ---- squares: bf16; ~10 pairs on ACT, 22 on DVE ----
                    x2 = x2pool.tile([P, DC, BLOC], BF16, tag="x2",
                                     name=f"x2_{k}")
                    for p in range(DC // 2):
                        src = x_sbs[k][:, 2 * p : 2 * p + 2]
                        dst = x2[:, 2 * p : 2 * p + 2]
                        on_act = (p == 1) or (p == 3 and k in (1, 3))
                        if on_act:
                            nc.scalar.activation(dst, src, AF.Square)
                        else:
                            nc.vector.tensor_mul(dst, src, src)

                    # ---- PE: mains j0-3, ss j0-3, mains j4-7, ss j4-7 ----
                    lg = plog.tile([C, BLOC], F32, tag="lg", name=f"lg{k}")

                    def mains(j0, j1):
                        for j in range(j0, j1):
                            nc.tensor.matmul(
                                lg[:], lhsT=w_sbs[k][:, j, :],
                                rhs=x_sbs[k][:, j],
                                start=(j == 0), stop=(j == DC - 1),
                                skip_group_check=True,
                            )

                    def ssmm(j0, j1):
                        for j in range(j0, j1):
                            nc.tensor.matmul(
                                ss[:],
                                lhsT=selk4_sb[:, i, :],
                                rhs=x2[:, j],
                                start=(i == 0 and j == 0),
                                stop=(i == HALF - 1 and j == DC - 1),
                                skip_group_check=True,
                            )

                    # whole-k blocks: 2 weight-set transitions per k instead
                    # of 4; k=0 runs mains first (they gate only on raw DMA)
                    if k == 0:
                        mains(0, DC // 2)
                        ssmm(0, DC // 2)
                        mains(DC // 2, DC)
                        ssmm(DC // 2, DC)
                    else:
                        ssmm(0, DC)
                        mains(0, DC)
                    lg_tiles[k] = lg

                    if k >= 1:
                        emit_drain(k - 1)

                    # half closed: f = sqrt(1/ss)
                    if i == HALF - 1:
                        recx = fxpool.tile([HALF, BLOC], F32, tag="recx",
                                           name=f"recx{half}")
                        scr = fxpool.tile([HALF, BLOC], F32, tag="rscr",
                                          name=f"rscr{half}")
                        nc.vector.reciprocal_approx_accurate(
                            recx[:], ss[:], scr[:]
                        )
                        fx = fxpool.tile([HALF, BLOC], F32R, tag="fx",
                                         name=f"fx{half}")
                        nc.scalar.activation(fx[:], recx[:], AF.Sqrt)
                        fx_sbs.append(fx)

                    if k >= HALF:
                        emit_output_stage(k - HALF)

                emit_drain(K - 1)
                for k in range(K):
                    if k not in o_done:
                        emit_output_stage(k)

    nc.compile()
    return nc


def _host_prep(pooled, active_expert_indices, weights, log_scales):
    idx = np.asarray(active_expert_indices).astype(np.int64)
    pooled = np.asarray(pooled, dtype=np.float32)
    weights = np.asarray(weights, dtype=np.float32)
    log_scales = np.asarray(log_scales, dtype=np.float32)

    # x: [B,K,D] -> bf16 -> per-core [K, P, DC, BLOC]  (k, d, j, b)
    pb = pooled.astype(NPBF16)
    xt_all = np.ascontiguousarray(
        pb.reshape(N_CORES, BLOC, K, DC, P).transpose(0, 2, 4, 3, 1)
    )
    # w: gather -> bf16 matmul layout; rwt folds scale / ||w|| (match the
    # bf16-rounded w the device multiplies with)
    wg = weights[idx].astype(NPBF16)  # [K, C, D]
    wt = np.ascontiguousarray(wg.reshape(K, C, DC, P).transpose(0, 3, 2, 1))
    s = np.minimum(np.exp(log_scales[idx]), 100.0).astype(np.float32)
    wn = np.sqrt((wg.astype(np.float32) ** 2).sum(-1))  # [K, C]
    rwt = np.ascontiguousarray((s[:, None] / np.maximum(wn, 1e-12)).T)

    selk4 = np.zeros((P, HALF, HALF), NPBF16)
    for i in range(HALF):
        selk4[:, i, i] = 1.0
    selc4 = np.zeros((HALF, HALF, C), np.float32)
    for i in range(HALF):
        selc4[i, i, :] = 1.0

    shared = {"wt": wt, "rwt": rwt, "selk4": selk4, "selc4": selc4}
    return [dict(shared, xt=np.ascontiguousarray(xt_all[co]))
            for co in range(N_CORES)]


def kernel(pooled, active_expert_indices, weights, log_scales):
    global LAST_RESULT, LAST_WALL_NS
    if "nc" not in _CACHE:
        _CACHE["nc"] = _build()
    nc = _CACHE["nc"]

    in_maps = _host_prep(pooled, active_expert_indices, weights, log_scales)

    t0 = time.perf_counter_ns()
    res = run_bass_kernel_spmd(nc, in_maps, core_ids=list(range(N_CORES)))
    LAST_WALL_NS = time.perf_counter_ns() - t0
    LAST_RESULT = res

    full = np.stack([res.results[co]["out"] for co in range(N_CORES)])
    return np.ascontiguousarray(
        full.transpose(0, 3, 1, 2).reshape(B, K, C)
    ).astype(np.float32)


# revision 25
# speedup vs baseline: 1.0770x; 1.0638x over previous
"""Trainium2 Bass kernel for nn_ExpertClassifierBank.

Computes, for pooled [B,K,D], expert weights [E,C,D], indices [K], log_scales [E]:
    x = l2norm(pooled, axis=-1)
    w = l2norm(weights[idx], axis=-1)
    out[b,k,c] = min(exp(log_scales[idx[k]]), 100) * dot(x[b,k], w[k,c])

Sharding: data-parallel over batch B across 8 NeuronCores (512 rows each);
the gathered expert weight bank is replicated.

Device algorithm per core (B_loc=512, K=8, D=1024=8x128, C=100):
  - host pre-transposes x and w to [k, d-part, j, *] bf16 tiles so the
    contraction dim d sits on SBUF partitions; host also folds the tiny
    per-expert constants into rwt[c,k] = min(exp(ls_k),100)/||w_k,c||
    (K*C values, same indexing-adjacent class as the expert gather).
  - main matmuls produce lg[k] = w_k^T x in PSUM; the drain to SBUF applies
    rwt as a per-partition scale (ACT Copy); the x-normalizer
    f = rsqrt(||x||^2) is broadcast over C partitions by a tiny selector
    matmul (selc4, f32r) and multiplied in on the DVE with bf16 output
    (halves output traffic).
  - row sums-of-squares: bf16 squares (split DVE 2x-packed / ACT) stream
    through the PE with one-hot k-slot selectors into a [4,BLOC] PSUM
    accumulator per half; recip+sqrt give f.
  - one ACT table set (sqrt_and_others: Square/Sqrt/Copy) loaded at t~0 by
    a dummy op.
  - x streams on the sync HWDGE queue from t=0 (k0 quartered, k1-3 halved
    for fast rampup); w + consts lead the gpsimd SWDGE queue; outputs k<4
    go out mid-kernel on gpsimd, k>=4 on the sync queue tail.  Per k the
    PE runs mains j0-3, ss j0-3, mains j4-7, ss j4-7; drains run one k
    behind, the f/output stage four k behind (gated by the half's f tile).
"""

import time

import numpy as np
import ml_dtypes

import concourse.bass as bass
import concourse.mybir as mybir
import concourse.tile as tile
from concourse import bacc
from concourse.bass_utils import run_bass_kernel_spmd

N_CORES = 8
B, K, D, C, E = 4096, 8, 1024, 100, 16
BLOC = B // N_CORES  # 512
P = 128
DC = D // P  # 8 d-chunks
HALF = 4  # k-batch size for the f pipeline

F32 = mybir.dt.float32
F32R = mybir.dt.float32r
BF16 = mybir.dt.bfloat16
AF = mybir.ActivationFunctionType
NPBF16 = ml_dtypes.bfloat16

_CACHE = {}

LAST_RESULT = None
LAST_WALL_NS = None


def _build():
    nc = bacc.Bacc(
        "TRN2", target_bir_lowering=False, debug=False, num_devices=N_CORES
    )

    xt = nc.dram_tensor("xt", [K, P, DC, BLOC], BF16, kind="ExternalInput").ap()
    wt = nc.dram_tensor("wt", [K, P, DC, C], BF16, kind="ExternalInput").ap()
    rwt = nc.dram_tensor("rwt", [C, K], F32, kind="ExternalInput").ap()
    selk4 = nc.dram_tensor("selk4", [P, HALF, HALF], BF16, kind="ExternalInput").ap()
    selc4 = nc.dram_tensor("selc4", [HALF, HALF, C], F32R, kind="ExternalInput").ap()
    out = nc.dram_tensor("out", [K, C, BLOC], BF16, kind="ExternalOutput").ap()

    with tile.TileContext(nc) as tc:
        with (
            tc.tile_pool(name="const", bufs=1) as cpool,
            tc.tile_pool(name="xres", bufs=K) as xpool,
            tc.tile_pool(name="x2", bufs=3) as x2pool,
            tc.tile_pool(name="wres", bufs=3) as wpool,
            tc.tile_pool(name="small", bufs=1) as spool,
            tc.tile_pool(name="lgs", bufs=5) as lgspool,
            tc.tile_pool(name="osb", bufs=3) as opool,
            tc.tile_pool(name="fx", bufs=2) as fxpool,
        ):
            # dummy first ACT op: pulls the (single) table set load to t~0
            dum = spool.tile([1, 1], F32, name="dum")
            nc.vector.memset(dum[:], 1.0)
            dum2 = spool.tile([1, 1], F32, name="dum2")
            nc.scalar.activation(dum2[:], dum[:], AF.Square)

            # ---- w + consts on the gpsimd SWDGE queue (w0 first: it
            # gates the first main matmuls) ----
            w_sbs = [None] * K

            def gp_w(k):
                w1 = wpool.tile([P, DC, C], BF16, tag="w", name=f"w{k}")
                nc.gpsimd.dma_start(w1[:], wt[k])
                w_sbs[k] = w1

            # w0 leads the sync HWDGE queue (SWDGE is too slow for the
            # first-matmul gate); the rest ride gpsimd, self-throttled by
            # the 3-deep w pool rotation
            w0 = wpool.tile([P, DC, C], BF16, tag="w", name="w0")
            nc.sync.dma_start(w0[:], wt[0])
            w_sbs[0] = w0
            selk4_sb = cpool.tile([P, HALF, HALF], BF16)
            nc.gpsimd.dma_start(selk4_sb[:], selk4[:])
            gp_w(1)
            rwt_sb = cpool.tile([C, K], F32)
            nc.gpsimd.dma_start(rwt_sb[:], rwt[:])
            gp_w(2)
            selc4_sb = cpool.tile([HALF, HALF, C], F32R)
            nc.gpsimd.dma_start(selc4_sb[:], selc4[:])
            for k in range(3, K):
                gp_w(k)

            # ---- x on the sync HWDGE queue: k0 halved, rest whole-k
            # (fewer pushes = faster descriptor issue = faster DMA ramp) ----
            x_sbs = []
            for k in range(K):
                x_sb = xpool.tile([P, DC, BLOC], BF16, tag="x", name=f"x{k}")
                nsplit = 2 if k == 0 else 1
                step = DC // nsplit
                for q in range(nsplit):
                    nc.sync.dma_start(
                        x_sb[:, q * step : (q + 1) * step],
                        xt[k][:, q * step : (q + 1) * step],
                    )
                x_sbs.append(x_sb)

            with (
                tc.tile_pool(name="pss", bufs=2, space="PSUM") as pss,
                tc.tile_pool(name="plog", bufs=3, space="PSUM") as plog,
                tc.tile_pool(name="pfb", bufs=3, space="PSUM") as pfb,
            ):
                sss = []
                fx_sbs = []
                lg_tiles = {}
                o_done = set()

                # HAM warmup: ~1.3us of junk matmuls on w0 as soon as it
                # lands, so the PE clock gate opens before the real stream
                wu = pfb.tile([HALF, 400], F32, tag="fb", name="warmup")
                for _ in range(4):
                    nc.tensor.matmul(
                        wu[:], lhsT=selk4_sb[:, 0, :],
                        rhs=w_sbs[0][:, :4],
                        start=True, stop=True,
                        skip_group_check=True,
                    )

                def emit_drain(k):
                    lgs = lgspool.tile([C, BLOC], F32, tag="lgs",
                                       name=f"lgs{k}")
                    nc.scalar.activation(
                        lgs[:], lg_tiles[k][:], AF.Copy,
                        scale=rwt_sb[:, k : k + 1],
                    )
                    lg_tiles[k] = lgs

                def emit_output_stage(k):
                    """f-broadcast matmul + mul + out DMA (fx half ready)."""
                    half, i = divmod(k, HALF)
                    fb = pfb.tile([C, BLOC], F32, tag="fb", name=f"fb{k}")
                    nc.tensor.matmul(
                        fb[:],
                        lhsT=selc4_sb[:, i, :],
                        rhs=fx_sbs[half][:],
                        start=True, stop=True,
                        skip_group_check=True,
                    )
                    o_sb = opool.tile([C, BLOC], BF16, tag="osb", name=f"o{k}")
                    nc.vector.tensor_mul(o_sb[:], lg_tiles[k][:], fb[:])
                    if k < HALF:
                        nc.gpsimd.dma_start(out[k], o_sb[:])
                    else:
                        nc.sync.dma_start(out[k], o_sb[:])
                    o_done.add(k)

                for k in range(K):
                    half, i = divmod(k, HALF)
                    if i == 0:
                        ss = pss.tile([HALF, BLOC], F32, tag="ss",
                                      name=f"ss{half}")
                        sss.append(ss)
                    ss = sss[half]

                    # ---- squares: bf16; ~10 pairs on ACT, 22 on DVE ----
                    x2 = x2pool.tile([P, DC, BLOC], BF16, tag="x2",
                                     name=f"x2_{k}")
                    for p in range(DC // 2):
                        src = x_sbs[k][:, 2 * p : 2 * p + 2]
                        dst = x2[:, 2 * p : 2 * p + 2]
                        on_act = (p == 1) or (p == 3 and k in (1, 3))
                        if on_act:
                            nc.scalar.activation(dst, src, AF.Square)
                        else:
                            nc.vector.tensor_mul(dst, src, src)

                    # ---- PE: mains j0-3, ss j0-3, mains j4-7, ss j4-7 ----
                    lg = plog.tile([C, BLOC], F32, tag="lg", name=f"lg{k}")

                    def mains(j0, j1):
                        for j in range(j0, j1):
                            nc.tensor.matmul(
                                lg[:], lhsT=w_sbs[k][:, j, :],
                                rhs=x_sbs[k][:, j],
                                start=(j == 0), stop=(j == DC - 1),
                                skip_group_check=True,
                            )

                    def ssmm(j0, j1):
                        for j in range(j0, j1):
                            nc.tensor.matmul(
                                ss[:],
                                lhsT=selk4_sb[:, i, :],
                                rhs=x2[:, j],
                                start=(i == 0 and j == 0),
                                stop=(i == HALF - 1 and j == DC - 1),
                                skip_group_check=True,
                            )

                    # whole-k blocks: 2 weight-set transitions per k instead
                    # of 4; k=0 runs mains first (they gate only on raw DMA)
                    if k == 0:
                        mains(0, DC // 2)
                        ssmm(0, DC // 2)
                        mains(DC // 2, DC)
                        ssmm(DC // 2, DC)
                    else:
                        ssmm(0, DC)
                        mains(0, DC)
                    lg_tiles[k] = lg

                    if k >= 1:
                        emit_drain(k - 1)

                    # half closed: f = sqrt(1/ss)
                    if i == HALF - 1:
                        recx = fxpool.tile([HALF, BLOC], F32, tag="recx",
                                           name=f"recx{half}")
                        scr = fxpool.tile([HALF, BLOC], F32, tag="rscr",
                                          name=f"rscr{half}")
                        nc.vector.reciprocal_approx_accurate(
                            recx[:], ss[:], scr[:]
                        )
                        fx = fxpool.tile([HALF, BLOC], F32R, tag="fx",
                                         name=f"fx{half}")
                        nc.scalar.activation(fx[:], recx[:], AF.Sqrt)
                        fx_sbs.append(fx)

                    if k >= HALF:
                        emit_output_stage(k - HALF)

                emit_drain(K - 1)
                for k in range(K):
                    if k not in o_done:
                        emit_output_stage(k)

    nc.compile()
    return nc


def _host_prep(pooled, active_expert_indices, weights, log_scales):
    idx = np.asarray(active_expert_indices).astype(np.int64)
    pooled = np.asarray(pooled, dtype=np.float32)
    weights = np.asarray(weights, dtype=np.float32)
    log_scales = np.asarray(log_scales, dtype=np.float32)

    # x: [B,K,D] -> bf16 -> per-core [K, P, DC, BLOC]  (k, d, j, b)
    pb = pooled.astype(NPBF16)
    xt_all = np.ascontiguousarray(
        pb.reshape(N_CORES, BLOC, K, DC, P).transpose(0, 2, 4, 3, 1)
    )
    # w: gather -> bf16 matmul layout; rwt folds scale / ||w|| (match the
    # bf16-rounded w the device multiplies with)
    wg = weights[idx].astype(NPBF16)  # [K, C, D]
    wt = np.ascontiguousarray(wg.reshape(K, C, DC, P).transpose(0, 3, 2, 1))
    s = np.minimum(np.exp(log_scales[idx]), 100.0).astype(np.float32)
    wn = np.sqrt((wg.astype(np.float32) ** 2).sum(-1))  # [K, C]
    rwt = np.ascontiguousarray((s[:, None] / np.maximum(wn, 1e-12)).T)

    selk4 = np.zeros((P, HALF, HALF), NPBF16)
    for i in range(HALF):
        selk4[:, i, i] = 1.0
    selc4 = np.zeros((HALF, HALF, C), np.float32)
    for i in range(HALF):
        selc4[i, i, :] = 1.0

    shared = {"wt": wt, "rwt": rwt, "selk4": selk4, "selc4": selc4}
    return [dict(shared, xt=np.ascontiguousarray(xt_all[co]))
            for co in range(N_CORES)]


def kernel(pooled, active_expert_indices, weights, log_scales):
    global LAST_RESULT, LAST_WALL_NS
    if "nc" not in _CACHE:
        _CACHE["nc"] = _build()
    nc = _CACHE["nc"]

    in_maps = _host_prep(pooled, active_expert_indices, weights, log_scales)

    t0 = time.perf_counter_ns()
    res = run_bass_kernel_spmd(nc, in_maps, core_ids=list(range(N_CORES)))
    LAST_WALL_NS = time.perf_counter_ns() - t0
    LAST_RESULT = res

    full = np.stack([res.results[co]["out"] for co in range(N_CORES)])
    return np.ascontiguousarray(
        full.transpose(0, 3, 1, 2).reshape(B, K, C)
    ).astype(np.float32)
